# revision 1
# baseline (speedup 1.0000x reference)
"""Cross-graph attention (block-diagonal segment-local attention) on 8 trn2 cores.

Strategy: graphs (batch ids) are contiguous segments in the sorted
atom_batch / residue_batch arrays.  Attention is block-diagonal: atoms of
graph b attend only to residues of graph b.  We shard 4 graphs per core,
pad every graph to a fixed (AG atoms, RG residues) slot so all 8 cores run
one identical SPMD program, and compute per-graph attention with no masks:

  - inputs are packed host-side as transposed tiles atom_h^T (128, A_pad),
    residue_h^T (128, R_pad); zero padding makes padded K columns / V rows
    exactly 0.
  - scores are computed transposed,  S^T = K @ Q^T,  so every matmul takes
    naturally-laid-out operands (no on-device transposes anywhere).
  - all matmuls run in float32r (fast fp32 mode, 1 cycle/row at free>=256).
  - exp(S/sqrt(128) + bias) is one ACT instruction per tile; the per-partition
    bias is 0 for real residues and -30000 for padded ones, so padded
    residues contribute exp = 0 downstream (mask costs zero instructions).
  - V is augmented with a ones column; U = expS^T.T @ [V | 1 | pad] then
    yields both the unnormalized context and the softmax denominator.
  - normalization + residual add run host-side: out = atom_h + U[:, :128]/U[:, 128:129].
"""

import sys

if "/opt/trn_rl_repo" not in sys.path:
    sys.path.insert(0, "/opt/trn_rl_repo")

import numpy as np

import concourse.bass as bass
import concourse.tile as tile
from concourse import bacc, mybir
from concourse.bass_utils import run_bass_kernel_spmd

N_CORES = 8
B = 32                      # number of graphs
P = 128                     # partitions
DH = 128                    # feature dims (DA == DR == DH == 128)
VW = 256                    # U-matmul rhs width (>=256 keeps f32r at full rate)
SCALE = 1.0 / np.sqrt(128.0)
NEG_BIAS = -30000.0

_kernel_cache: dict = {}


def _col_chunks(n):
    """Split n columns into matmul chunks of <=512 that never cross a
    512-element PSUM bank boundary (matmul output must stay in one bank)."""
    out, i = [], 0
    while i < n:
        w = min(512, n - i)
        out.append((i, w))
        i += w
    return out


def _build_kernel(AG: int, RG: int, G: int):
    """One SPMD program: G graph slots of (AG atoms, RG residues) per core."""
    A_pad = G * AG
    R_pad = G * RG
    nkg = RG // P               # residue chunks per graph
    nRc = G * nkg               # residue chunks per core
    ntg = AG // P               # atom chunks per graph
    nAc = G * ntg               # atom chunks per core
    f32 = mybir.dt.float32
    f32r = mybir.dt.float32r

    nc = bacc.Bacc("TRN2")
    atomT = nc.dram_tensor("atomT", [P, A_pad], f32r, kind="ExternalInput")
    resT = nc.dram_tensor("resT", [P, R_pad], f32r, kind="ExternalInput")
    wqT = nc.dram_tensor("wqT", [P, DH], f32r, kind="ExternalInput")
    wkT = nc.dram_tensor("wkT", [P, DH], f32r, kind="ExternalInput")
    wvT = nc.dram_tensor("wvT", [P, DH], f32r, kind="ExternalInput")
    bias = nc.dram_tensor("bias", [P, nRc], f32, kind="ExternalInput")
    out = nc.dram_tensor("out", [A_pad, DH + 1], f32, kind="ExternalOutput")

    sg_chunks = _col_chunks(AG)

    with tile.TileContext(nc) as tc:
        with (
            tc.tile_pool(name="singles", bufs=1) as singles,
            tc.tile_pool(name="psum_big", bufs=3, space="PSUM") as ps_big,
            tc.tile_pool(name="psum_small", bufs=2, space="PSUM") as ps_small,
        ):
            # ---- load everything to SBUF ----
            atomT_sb = singles.tile([P, A_pad], f32r)
            resT_sb = singles.tile([P, R_pad], f32r)
            wqT_sb = singles.tile([P, DH], f32r)
            wkT_sb = singles.tile([P, DH], f32r)
            wvT_sb = singles.tile([P, VW], f32r)
            bias_sb = singles.tile([P, nRc], f32)
            nc.sync.dma_start(wqT_sb[:], wqT[:])
            nc.sync.dma_start(wkT_sb[:], wkT[:])
            nc.vector.memset(wvT_sb[:].bitcast(f32), 0.0)
            nc.sync.dma_start(wvT_sb[:, :DH], wvT[:])
            nc.sync.dma_start(bias_sb[:], bias[:])
            # chunked loads so compute can start on the first chunk
            for i in range(0, R_pad, 512):
                w = min(512, R_pad - i)
                nc.sync.dma_start(resT_sb[:, i : i + w], resT[:, i : i + w])
            for i in range(0, A_pad, 512):
                w = min(512, A_pad - i)
                nc.sync.dma_start(atomT_sb[:, i : i + w], atomT[:, i : i + w])

            # V' = [residue_h @ W_v^T | 1 | junk] laid out per residue chunk
            V_sb = singles.tile([P, nRc, VW], f32r)
            nc.vector.memset(V_sb[:].bitcast(f32), 1.0)

            # ---- Q^T = W_q @ atom_h^T, K^T = W_k @ residue_h^T ----
            # psum->sbuf copies alternate DVE/ACT so neither engine gates PE
            def copy_alt(i, dst, src):
                eng = nc.vector if i % 2 == 0 else nc.scalar
                if eng is nc.vector:
                    eng.tensor_copy(dst, src)
                else:
                    eng.copy(dst, src)

            KT_sb = singles.tile([P, R_pad], f32r)
            for n, i in enumerate(range(0, R_pad, 512)):
                w = min(512, R_pad - i)
                pk = ps_big.tile([P, 512], f32, tag="big")
                nc.tensor.matmul(
                    pk[:, :w], wkT_sb[:], resT_sb[:, i : i + w],
                    start=True, stop=True,
                )
                copy_alt(n, KT_sb[:, i : i + w], pk[:, :w])

            QT_sb = singles.tile([P, A_pad], f32r)
            for n, i in enumerate(range(0, A_pad, 512)):
                w = min(512, A_pad - i)
                pq = ps_big.tile([P, 512], f32, tag="big")
                nc.tensor.matmul(
                    pq[:, :w], wqT_sb[:], atomT_sb[:, i : i + w],
                    start=True, stop=True,
                )
                copy_alt(n + 1, QT_sb[:, i : i + w], pq[:, :w])

            # ---- V chunks (rhs padded to VW cols so f32r runs at rate 1) ----
            for k in range(nRc):
                pv = ps_small.tile([P, VW], f32, tag="small")
                nc.tensor.matmul(
                    pv[:], resT_sb[:, k * P : (k + 1) * P], wvT_sb[:],
                    start=True, stop=True,
                )
                copy_alt(k, V_sb[:, k, :DH], pv[:, :DH])

            # ---- per-graph attention ----
            ES_sb = singles.tile([P, nRc, AG], f32r)   # exp(S^T) per residue chunk
            OUT_sb = singles.tile([P, nAc, DH + 1], f32)

            for g in range(G):
                a0 = g * AG
                for k in range(nkg):
                    kg = g * nkg + k
                    r0 = kg * P
                    ps = ps_big.tile([P, 512 * ((AG + 511) // 512)], f32, tag="big")
                    for c, w in sg_chunks:
                        nc.tensor.matmul(
                            ps[:, c : c + w],
                            KT_sb[:, r0 : r0 + P],
                            QT_sb[:, a0 + c : a0 + c + w],
                            start=True, stop=True,
                        )
                    nc.scalar.activation(
                        ES_sb[:, kg, :], ps[:, :AG],
                        mybir.ActivationFunctionType.Exp,
                        bias=bias_sb[:, kg : kg + 1], scale=SCALE,
                    )

                for t in range(ntg):
                    tg = g * ntg + t
                    pu = ps_small.tile([P, VW], f32, tag="small")
                    for k in range(nkg):
                        kg = g * nkg + k
                        nc.tensor.matmul(
                            pu[:],
                            ES_sb[:, kg, t * P : (t + 1) * P],
                            V_sb[:, kg, :],
                            start=(k == 0), stop=(k == nkg - 1),
                        )
                    nc.vector.tensor_copy(OUT_sb[:, tg, :], pu[:, : DH + 1])

                # stream this graph's rows out while later graphs compute
                nc.sync.dma_start(
                    out[g * AG : (g + 1) * AG, :].rearrange(
                        "(t p) f -> p t f", p=P
                    ),
                    OUT_sb[:, g * ntg : (g + 1) * ntg, :],
                )

    nc.compile()
    return nc


def kernel(atom_h, residue_h, atom_batch, residue_batch, W_q, W_k, W_v):
    atom_h = np.asarray(atom_h, dtype=np.float32)
    residue_h = np.asarray(residue_h, dtype=np.float32)
    atom_batch = np.asarray(atom_batch)
    residue_batch = np.asarray(residue_batch)
    W_q = np.asarray(W_q, dtype=np.float32)
    W_k = np.asarray(W_k, dtype=np.float32)
    W_v = np.asarray(W_v, dtype=np.float32)

    A = atom_h.shape[0]
    R = residue_h.shape[0]
    n_b = max(B, int(atom_batch.max()) + 1 if A else B,
              int(residue_batch.max()) + 1 if R else B)

    ac = np.bincount(atom_batch, minlength=n_b)
    rc = np.bincount(residue_batch, minlength=n_b)
    a_off = np.concatenate([[0], np.cumsum(ac)])
    r_off = np.concatenate([[0], np.cumsum(rc)])

    G = (n_b + N_CORES - 1) // N_CORES
    AG = max(P, int(np.ceil(ac.max() / P)) * P)
    RG = max(P, int(np.ceil(rc.max() / P)) * P)
    A_pad, R_pad = G * AG, G * RG
    nkg = RG // P
    nRc = G * nkg

    key = (AG, RG, G)
    if key not in _kernel_cache:
        _kernel_cache[key] = _build_kernel(AG, RG, G)
    nc = _kernel_cache[key]

    wqT = np.ascontiguousarray(W_q.T)
    wkT = np.ascontiguousarray(W_k.T)
    wvT = np.ascontiguousarray(W_v.T)

    in_maps = []
    for c in range(N_CORES):
        atomT_c = np.zeros((P, A_pad), dtype=np.float32)
        resT_c = np.zeros((P, R_pad), dtype=np.float32)
        bias_c = np.zeros((P, nRc), dtype=np.float32)
        for j in range(G):
            g = c * G + j
            if g >= n_b:
                bias_c[:, j * nkg : (j + 1) * nkg] = NEG_BIAS
                continue
            na, nr = int(ac[g]), int(rc[g])
            if na:
                atomT_c[:, j * AG : j * AG + na] = atom_h[a_off[g] : a_off[g] + na].T
            if nr:
                resT_c[:, j * RG : j * RG + nr] = residue_h[r_off[g] : r_off[g] + nr].T
            flat = np.full(RG, NEG_BIAS, dtype=np.float32)
            flat[:nr] = 0.0
            bias_c[:, j * nkg : (j + 1) * nkg] = flat.reshape(nkg, P).T
        in_maps.append({
            "atomT": atomT_c, "resT": resT_c,
            "wqT": wqT, "wkT": wkT, "wvT": wvT,
            "bias": bias_c,
        })

    res = run_bass_kernel_spmd(nc, in_maps, core_ids=list(range(N_CORES)))

    result = atom_h.copy()
    for c in range(N_CORES):
        u = res.results[c]["out"]
        for j in range(G):
            g = c * G + j
            if g >= n_b:
                continue
            na, nr = int(ac[g]), int(rc[g])
            if na == 0 or nr == 0:
                continue
            rows = u[j * AG : j * AG + na]
            result[a_off[g] : a_off[g] + na] += rows[:, :DH] / rows[:, DH : DH + 1]
    return result



# revision 18
# speedup vs baseline: 1.5874x; 1.5874x over previous
"""Cross-graph attention (block-diagonal segment-local attention) on 8 trn2 cores.

Strategy: graphs are contiguous segments in the sorted batch arrays; attention
is block-diagonal.  4 graphs per core; the 32 graphs are grouped host-side
into 4 slot-groups of 8 similarly-sized graphs (one per core), and each slot
has its own padded (AG_j, RG_j) so all 8 cores run one identical SPMD program
with minimal padding.

Design:
  - W_q is folded into W_k host-side (W_kq = W_k^T W_q / sqrt(DH)):
    scores^T = T1^T-contraction with raw atom columns; no Q stage on device.
  - everything streams in bf16 (half the DMA bytes; matmuls run at
    1 cycle/row at any output width, so V/U are exactly 129 wide).
  - masking via the V ones-column: padded residues get V rows = 0 and
    ones-col = 0 (host-packed), so their exp(0)=1 contributes nothing to
    numerator or denominator.  No ACT bias, so exp instructions merge
    across residue chunks (one ACT per <=2 chunks).
  - a dummy warmup matmul at t~0 starts the PE p-state ramp clock.
  - U/V psum outputs are packed 3-per-bank; DVE evacuates all PSUM
    (Pool/GPSIMD cannot touch PSUM on HW); ACT does only the exp.
  - output DMAs are spread across SP / Pool(SWDGE) / ACT queues; the last
    slot is the cheapest one and its output is split so the exposed tail
    transfer is short.
  - normalization + residual add run host-side: out = atom_h + U[:, :128]/U[:, 128].
"""

import sys

if "/opt/trn_rl_repo" not in sys.path:
    sys.path.insert(0, "/opt/trn_rl_repo")

import ml_dtypes
import numpy as np

import concourse.bass as bass
import concourse.tile as tile
from concourse import bacc, mybir
from concourse.bass_utils import run_bass_kernel_spmd

N_CORES = 8
B = 32                      # number of graphs
P = 128                     # partitions
DH = 128                    # feature dims (DA == DR == DH == 128)
SCALE = 1.0 / np.sqrt(128.0)

BF16 = ml_dtypes.bfloat16

_kernel_cache: dict = {}


def _bank_chunks(lo, hi):
    """Split [lo, hi) into matmul chunks that never cross a 512-col PSUM
    bank boundary."""
    out, i = [], lo
    while i < hi:
        nxt = min(hi, (i // 512 + 1) * 512)
        out.append((i, nxt - i))
        i = nxt
    return out


def _build_kernel(slots):
    """One SPMD program; ``slots`` is a tuple of (ntg, nkg) per graph slot:
    slot j holds one graph of <= ntg*128 atoms / nkg*128 residues."""
    G = len(slots)
    ntgs = [s[0] for s in slots]
    nkgs = [s[1] for s in slots]
    AGs = [t * P for t in ntgs]
    aoffs = np.concatenate([[0], np.cumsum(AGs)]).astype(int)
    koffs = np.concatenate([[0], np.cumsum(nkgs)]).astype(int)
    A_pad = int(aoffs[-1])
    nRc = int(koffs[-1])
    R_pad = nRc * P
    f32 = mybir.dt.float32
    bf16 = mybir.dt.bfloat16

    nc = bacc.Bacc("TRN2")
    atomT = nc.dram_tensor("atomT", [P, A_pad], bf16, kind="ExternalInput")
    resT = nc.dram_tensor("resT", [P, R_pad], bf16, kind="ExternalInput")
    wkq = nc.dram_tensor("wkq", [P, DH], bf16, kind="ExternalInput")
    wvT = nc.dram_tensor("wvT", [P, DH], bf16, kind="ExternalInput")
    onesc = nc.dram_tensor("onesc", [P, nRc], bf16, kind="ExternalInput")
    out = nc.dram_tensor("out", [A_pad, DH + 1], f32, kind="ExternalOutput")

    def kgroups(nkg):
        ks, k = [], 0
        while k < nkg:
            ks.append((k, min(2, nkg - k)))
            k += 2
        return ks

    with tile.TileContext(nc) as tc:
        with (
            tc.tile_pool(name="singles", bufs=1) as singles,
            tc.tile_pool(name="ps_sc", bufs=2, space="PSUM") as ps_sc,
            tc.tile_pool(name="ps_u", bufs=2, space="PSUM") as ps_u,
        ):
            atomT_sb = singles.tile([P, A_pad], bf16)
            resT_sb = singles.tile([P, R_pad], bf16)
            wkq_sb = singles.tile([P, DH], bf16)
            wvT_sb = singles.tile([P, DH + 1], bf16)
            onesc_sb = singles.tile([P, nRc], bf16)
            T1T_sb = singles.tile([P, R_pad], bf16)
            V_sb = singles.tile([P, nRc, DH + 1], bf16)
            warm_sb = singles.tile([P, 2], bf16)
            ES_sb = [singles.tile([P, nkgs[g], AGs[g]], bf16, name=f"es{g}")
                     for g in range(G)]
            OB_sb = [singles.tile([P, ntgs[g], DH + 1], f32, name=f"ob{g}")
                     for g in range(G)]

            # ---- PE warmup: start the p-state ramp clock immediately ----
            nc.gpsimd.memset(warm_sb[:], 0.0)
            pw = ps_u.tile([P, 3, DH + 1], f32, tag="u")
            nc.tensor.matmul(pw[:2, 0, :1], warm_sb[:], warm_sb[:, :1],
                             start=True, stop=True)

            nc.vector.memset(wvT_sb[:, DH : DH + 1], 0.0)

            # ---- input DMAs (SP/HWDGE), in first-use order ----
            nc.sync.dma_start(wkq_sb[:], wkq[:])
            nc.sync.dma_start(resT_sb[:, :512], resT[:, :512])
            if R_pad > 512:
                nc.sync.dma_start(resT_sb[:, 512:], resT[:, 512:])
            nc.sync.dma_start(atomT_sb[:, : AGs[0]], atomT[:, : AGs[0]])
            nc.sync.dma_start(wvT_sb[:, :DH], wvT[:])
            nc.sync.dma_start(onesc_sb[:], onesc[:])
            nc.sync.dma_start(atomT_sb[:, AGs[0] :], atomT[:, AGs[0] :])

            # ---- T1^T = contraction of W_kq with res^T, 512-col chunks ----
            for i in range(0, R_pad, 512):
                w = min(512, R_pad - i)
                pt = ps_sc.tile([P, 1280], f32, tag="sc")
                nc.tensor.matmul(pt[:, :w], wkq_sb[:], resT_sb[:, i : i + w],
                                 start=True, stop=True)
                nc.vector.tensor_copy(T1T_sb[:, i : i + w], pt[:, :w])

            # ---- V rows per graph, packed 3 per psum bank ----
            for g in range(G):
                k0, nkg = int(koffs[g]), nkgs[g]
                pv = ps_u.tile([P, 3, DH + 1], f32, tag="u")
                for j in range(nkg):
                    nc.tensor.matmul(pv[:, j, :],
                                     resT_sb[:, (k0 + j) * P : (k0 + j + 1) * P],
                                     wvT_sb[:], start=True, stop=True)
                nc.vector.tensor_copy(V_sb[:, k0 : k0 + nkg, :], pv[:, :nkg, :])
                # ones-column (0 for padded residues) overwrites the junk col
                nc.gpsimd.tensor_copy(V_sb[:, k0 : k0 + nkg, DH],
                                      onesc_sb[:, k0 : k0 + nkg])

            # ---- per-graph attention, ACT-paced pipeline ----
            def emit_scores(g):
                a0, k0 = int(aoffs[g]), int(koffs[g])
                AG, nkg = AGs[g], nkgs[g]
                es = ES_sb[g]
                for kk, kn in kgroups(nkg):
                    ps = ps_sc.tile([P, 1280], f32, tag="sc")
                    for j in range(kn):
                        kg = k0 + kk + j
                        for c, w in _bank_chunks(j * AG, (j + 1) * AG):
                            nc.tensor.matmul(
                                ps[:, c : c + w],
                                T1T_sb[:, kg * P : (kg + 1) * P],
                                atomT_sb[:, a0 + c - j * AG : a0 + c - j * AG + w],
                                start=True, stop=True,
                            )
                    nc.scalar.activation(
                        es[:, kk : kk + kn, :], ps[:, : kn * AG],
                        mybir.ActivationFunctionType.Exp,
                    )

            def emit_u(g):
                a0, k0 = int(aoffs[g]), int(koffs[g])
                ntg, nkg = ntgs[g], nkgs[g]
                es, ob = ES_sb[g], OB_sb[g]
                for t0 in range(0, ntg, 3):
                    tn = min(3, ntg - t0)
                    pu = ps_u.tile([P, 3, DH + 1], f32, tag="u")
                    for t in range(t0, t0 + tn):
                        for k in range(nkg):
                            nc.tensor.matmul(
                                pu[:, t - t0, :],
                                es[:, k, t * P : (t + 1) * P],
                                V_sb[:, k0 + k, :],
                                start=(k == 0), stop=(k == nkg - 1),
                            )
                    nc.vector.tensor_copy(ob[:, t0 : t0 + tn, :], pu[:, :tn, :])
                # stream this graph's rows out while later graphs compute
                if g < G - 1:
                    eng = nc.sync if g % 2 == 0 else nc.gpsimd
                    eng.dma_start(
                        out[a0 : a0 + ntg * P, :].rearrange(
                            "(t p) f -> p t f", p=P
                        ),
                        ob[:],
                    )
                else:
                    # last slot: head DMA needs only the first copy group;
                    # the exposed tail transfer is short
                    split = max(1, min(3, ntg - 1))
                    nc.sync.dma_start(
                        out[a0 : a0 + split * P, :].rearrange(
                            "(t p) f -> p t f", p=P
                        ),
                        ob[:, :split, :],
                    )
                    nc.scalar.dma_start(
                        out[a0 + split * P : a0 + ntg * P, :].rearrange(
                            "(t p) f -> p t f", p=P
                        ),
                        ob[:, split:, :],
                    )

            emit_scores(0)
            for g in range(1, G):
                emit_scores(g)
                emit_u(g - 1)
            emit_u(G - 1)

    nc.compile()
    return nc


def _pack_slots(ac, rc, n_cores):
    """Group graphs into slots of ``n_cores`` similarly-shaped graphs.
    Returns (slots, assign) where slots[j] = (ntg, nkg) and assign[j] is the
    list of graph ids in slot j (one per core), ordered by slot cost desc."""
    a_ch = np.maximum(1, np.ceil(ac / P).astype(int))
    r_ch = np.maximum(1, np.ceil(rc / P).astype(int))
    from collections import defaultdict

    buckets = defaultdict(list)
    for g in range(len(ac)):
        buckets[(int(a_ch[g]), int(r_ch[g]))].append(g)

    slot_groups = []
    # pure same-shape slots first (cost order), then leftovers by cost
    for shape in sorted(buckets, key=lambda s: -(s[0] * s[1])):
        while len(buckets[shape]) >= n_cores:
            slot_groups.append([buckets[shape].pop() for _ in range(n_cores)])
    leftovers = [g for shape in sorted(buckets, key=lambda s: -(s[0] * s[1]))
                 for g in buckets[shape]]
    while leftovers:
        slot_groups.append(leftovers[:n_cores])
        leftovers = leftovers[n_cores:]
    # pad the final partial group by reusing empty pseudo-graphs (none here
    # for 32 graphs / 8 cores, but keep it safe)
    slots = []
    for grp in slot_groups:
        ntg = int(max(a_ch[g] for g in grp))
        nkg = int(max(r_ch[g] for g in grp))
        slots.append((ntg, nkg))
    # order slots by exp work desc so the cheapest slot is processed last
    order = sorted(range(len(slots)),
                   key=lambda j: -(slots[j][0] * slots[j][1]))
    slots = [slots[j] for j in order]
    slot_groups = [slot_groups[j] for j in order]
    return slots, slot_groups


def kernel(atom_h, residue_h, atom_batch, residue_batch, W_q, W_k, W_v):
    atom_h = np.asarray(atom_h, dtype=np.float32)
    residue_h = np.asarray(residue_h, dtype=np.float32)
    atom_batch = np.asarray(atom_batch)
    residue_batch = np.asarray(residue_batch)
    W_q = np.asarray(W_q, dtype=np.float32)
    W_k = np.asarray(W_k, dtype=np.float32)
    W_v = np.asarray(W_v, dtype=np.float32)

    A = atom_h.shape[0]
    R = residue_h.shape[0]
    n_b = max(B, int(atom_batch.max()) + 1 if A else B,
              int(residue_batch.max()) + 1 if R else B)

    ac = np.bincount(atom_batch, minlength=n_b)
    rc = np.bincount(residue_batch, minlength=n_b)
    a_off = np.concatenate([[0], np.cumsum(ac)])
    r_off = np.concatenate([[0], np.cumsum(rc)])

    slots, slot_groups = _pack_slots(ac, rc, N_CORES)
    G = len(slots)
    ntgs = [s[0] for s in slots]
    nkgs = [s[1] for s in slots]
    AGs = [t * P for t in ntgs]
    aoffs = np.concatenate([[0], np.cumsum(AGs)]).astype(int)
    koffs = np.concatenate([[0], np.cumsum(nkgs)]).astype(int)
    A_pad = int(aoffs[-1])
    nRc = int(koffs[-1])
    R_pad = nRc * P

    key = tuple(slots)
    if key not in _kernel_cache:
        _kernel_cache[key] = _build_kernel(key)
    nc = _kernel_cache[key]

    # host-side weight prep: fold W_q and the softmax scale into W_k
    wkq = ((W_k.T @ W_q) * SCALE).astype(BF16)        # [i, j]
    wvT = np.ascontiguousarray(W_v.T).astype(BF16)    # [i, o]

    in_maps = []
    for c in range(N_CORES):
        atomT_c = np.zeros((P, A_pad), dtype=BF16)
        resT_c = np.zeros((P, R_pad), dtype=BF16)
        onesc_c = np.zeros((P, nRc), dtype=BF16)
        for j in range(G):
            if c >= len(slot_groups[j]):
                continue
            g = slot_groups[j][c]
            na, nr = int(ac[g]), int(rc[g])
            RGj = nkgs[j] * P
            if na:
                atomT_c[:, aoffs[j] : aoffs[j] + na] = (
                    atom_h[a_off[g] : a_off[g] + na].T.astype(BF16))
            if nr:
                resT_c[:, koffs[j] * P : koffs[j] * P + nr] = (
                    residue_h[r_off[g] : r_off[g] + nr].T.astype(BF16))
            flat = np.zeros(RGj, dtype=BF16)
            flat[:nr] = 1.0
            onesc_c[:, koffs[j] : koffs[j + 1]] = flat.reshape(nkgs[j], P).T
        in_maps.append({
            "atomT": atomT_c, "resT": resT_c,
            "wkq": wkq, "wvT": wvT, "onesc": onesc_c,
        })

    res = run_bass_kernel_spmd(nc, in_maps, core_ids=list(range(N_CORES)))

    result = atom_h.copy()
    for c in range(N_CORES):
        u = res.results[c]["out"]
        for j in range(G):
            if c >= len(slot_groups[j]):
                continue
            g = slot_groups[j][c]
            na, nr = int(ac[g]), int(rc[g])
            if na == 0 or nr == 0:
                continue
            rows = u[aoffs[j] : aoffs[j] + na]
            result[a_off[g] : a_off[g] + na] += rows[:, :DH] / rows[:, DH : DH + 1]
    return result


# revision 42
# speedup vs baseline: 1.6060x; 1.0117x over previous
"""Cross-graph attention (block-diagonal segment-local attention) on 8 trn2 cores.

Strategy: graphs are contiguous segments in the sorted batch arrays; attention
is block-diagonal.  4 graphs per core; the 32 graphs are grouped host-side
into 4 slot-groups of 8 similarly-sized graphs (one per core), and each slot
has its own padded (AG_j, RG_j) so all 8 cores run one identical SPMD program
with minimal padding.

Design:
  - W_q is folded into W_k host-side (W_kq = W_k^T W_q / sqrt(DH)):
    scores^T = T1^T-contraction with raw atom columns; no Q stage on device.
  - everything streams in bf16 (half the DMA bytes; matmuls run at
    1 cycle/row at any output width, so V/U are exactly 129 wide).
  - masking via the V ones-column: padded residues get V rows = 0 and
    ones-col = 0 (host-packed), so their exp(0)=1 contributes nothing to
    numerator or denominator.  No ACT bias, so exp instructions merge
    across residue chunks (one ACT per <=2 chunks).
  - a dummy warmup matmul at t~0 starts the PE p-state ramp clock.
  - U/V psum outputs are packed 3-per-bank; DVE evacuates all PSUM
    (Pool/GPSIMD cannot touch PSUM on HW); ACT does only the exp.
  - output DMAs are spread across SP / Pool(SWDGE) / ACT queues; the last
    slot is the cheapest one and its output is split so the exposed tail
    transfer is short.
  - normalization + residual add run host-side: out = atom_h + U[:, :128]/U[:, 128].
"""

import sys

if "/opt/trn_rl_repo" not in sys.path:
    sys.path.insert(0, "/opt/trn_rl_repo")

import ml_dtypes
import numpy as np

import concourse.bass as bass
import concourse.tile as tile
from concourse import bacc, mybir
from concourse.bass_utils import run_bass_kernel_spmd

N_CORES = 8
B = 32                      # number of graphs
P = 128                     # partitions
DH = 128                    # feature dims (DA == DR == DH == 128)
SCALE = 1.0 / np.sqrt(128.0)

BF16 = ml_dtypes.bfloat16

_kernel_cache: dict = {}


def _bank_chunks(lo, hi):
    """Split [lo, hi) into matmul chunks that never cross a 512-col PSUM
    bank boundary."""
    out, i = [], lo
    while i < hi:
        nxt = min(hi, (i // 512 + 1) * 512)
        out.append((i, nxt - i))
        i = nxt
    return out


def _build_kernel(slots):
    """One SPMD program; ``slots`` is a tuple of (ntg, nkg) per graph slot:
    slot j holds one graph of <= ntg*128 atoms / nkg*128 residues."""
    G = len(slots)
    ntgs = [s[0] for s in slots]
    nkgs = [s[1] for s in slots]
    AGs = [t * P for t in ntgs]
    aoffs = np.concatenate([[0], np.cumsum(AGs)]).astype(int)
    koffs = np.concatenate([[0], np.cumsum(nkgs)]).astype(int)
    A_pad = int(aoffs[-1])
    nRc = int(koffs[-1])
    R_pad = nRc * P
    f32 = mybir.dt.float32
    bf16 = mybir.dt.bfloat16

    nc = bacc.Bacc("TRN2")
    # head = [wkq(128) | res chunk0(128) | wvT+zerocol(129) | onesc(nRc)]
    HW_ = DH + P + (DH + 1) + nRc
    atomT = nc.dram_tensor("atomT", [P, A_pad], bf16, kind="ExternalInput")
    resT = nc.dram_tensor("resT", [P, R_pad], bf16, kind="ExternalInput")
    head = nc.dram_tensor("head", [P, HW_], bf16, kind="ExternalInput")
    out = nc.dram_tensor("out", [A_pad, DH + 1], f32, kind="ExternalOutput")

    with tile.TileContext(nc) as tc:
        with (
            tc.tile_pool(name="singles", bufs=1) as singles,
            tc.tile_pool(name="ps_sc", bufs=3, space="PSUM") as ps_sc,
            tc.tile_pool(name="ps_u", bufs=2, space="PSUM") as ps_u,
        ):
            atomT_sb = singles.tile([P, A_pad], bf16)
            resT_sb = singles.tile([P, R_pad], bf16)
            head_sb = singles.tile([P, HW_], bf16)
            T1T_sb = singles.tile([P, R_pad], bf16)
            V_sb = singles.tile([P, nRc, DH + 1], bf16)
            warm_sb = singles.tile([P, 2], bf16)
            wkq_sb = head_sb[:, :DH]
            wvT_sb = head_sb[:, DH + P : DH + P + DH + 1]
            onesc_sb = head_sb[:, DH + P + DH + 1 :]

            def res_chunk(kg):
                # residue chunk 0 rides in the head DMA
                if kg == 0:
                    return head_sb[:, DH : DH + P]
                return resT_sb[:, kg * P : (kg + 1) * P]
            ES_sb = [singles.tile([P, nkgs[g], AGs[g]], bf16, name=f"es{g}")
                     for g in range(G)]
            OB_sb = [singles.tile([P, ntgs[g], DH + 1], f32, name=f"ob{g}")
                     for g in range(G)]

            # ---- PE warmup: start the p-state ramp clock immediately ----
            nc.gpsimd.memset(warm_sb[:], 0.0)
            pw = ps_u.tile([P, 512], f32, tag="u")
            nc.tensor.matmul(pw[:2, :1], warm_sb[:], warm_sb[:, :1],
                             start=True, stop=True)

            # ---- input DMAs (SP/HWDGE), in first-use order ----
            nc.sync.dma_start(head_sb[:], head[:])
            nc.sync.dma_start(atomT_sb[:, : AGs[0]], atomT[:, : AGs[0]])
            nc.sync.dma_start(resT_sb[:, P : min(512, R_pad)],
                              resT[:, P : min(512, R_pad)])
            if R_pad > 512:
                nc.sync.dma_start(resT_sb[:, 512:], resT[:, 512:])
            nc.sync.dma_start(atomT_sb[:, AGs[0] :], atomT[:, AGs[0] :])
            # (atom-rest is last: graph-1+ scores start well after it lands)

            # ---- T1^T = contraction of W_kq with res^T ----
            # Piece list: per-residue-chunk pieces for graph 0 (so each
            # scores k starts as soon as its own residues land), then
            # 512-col chunks.  Emitted lazily via ensure_t1t.
            nkg0 = nkgs[0]
            pieces = [(kg * P, P) for kg in range(nkg0)]
            i = nkg0 * P
            while i < R_pad:
                w = min(512, R_pad - i)
                pieces.append((i, w))
                i += w
            t1_next = [0]    # next piece index to emit

            def ensure_t1t(upto):
                """Emit T1T pieces until residue columns [0, upto) covered."""
                while t1_next[0] < len(pieces):
                    lo, w = pieces[t1_next[0]]
                    if lo >= upto:
                        break
                    src = res_chunk(0) if (lo, w) == (0, P) else \
                        resT_sb[:, lo : lo + w]
                    pt = ps_u.tile([P, 512], f32, tag="u")
                    nc.tensor.matmul(pt[:, :w], wkq_sb, src,
                                     start=True, stop=True)
                    nc.vector.tensor_copy(T1T_sb[:, lo : lo + w], pt[:, :w])
                    t1_next[0] += 1

            # ---- V rows per graph, packed 3 per psum bank ----
            def emit_v(g):
                k0, nkg = int(koffs[g]), nkgs[g]
                pv = ps_u.tile([P, 512], f32, tag="u")
                for j in range(nkg):
                    nc.tensor.matmul(pv[:, j * (DH + 1) : (j + 1) * (DH + 1)],
                                     res_chunk(k0 + j),
                                     wvT_sb, start=True, stop=True)
                nc.vector.tensor_copy(
                    V_sb[:, k0 : k0 + nkg, :],
                    pv[:, : nkg * (DH + 1)].rearrange("p (k f) -> p k f", k=nkg))
                # ones-column (0 for padded residues) overwrites the junk col
                nc.gpsimd.tensor_copy(V_sb[:, k0 : k0 + nkg, DH],
                                      onesc_sb[:, k0 : k0 + nkg])

            # ---- per-graph attention, ACT-paced pipeline ----
            def emit_scores(g):
                a0, k0 = int(aoffs[g]), int(koffs[g])
                AG, nkg = AGs[g], nkgs[g]
                es = ES_sb[g]
                for k in range(nkg):
                    # stay 3 residue chunks ahead so T1T copies are never on
                    # the scores critical path
                    ensure_t1t(min(R_pad, (k0 + k + 3) * P))
                    ps = ps_sc.tile([P, 640], f32, tag="sc")
                    for c, w in _bank_chunks(0, AG):
                        nc.tensor.matmul(
                            ps[:, c : c + w],
                            T1T_sb[:, (k0 + k) * P : (k0 + k + 1) * P],
                            atomT_sb[:, a0 + c : a0 + c + w],
                            start=True, stop=True,
                        )
                    nc.scalar.activation(
                        es[:, k, :], ps[:, :AG],
                        mybir.ActivationFunctionType.Exp,
                    )

            def emit_u(g):
                a0, k0 = int(aoffs[g]), int(koffs[g])
                ntg, nkg = ntgs[g], nkgs[g]
                es, ob = ES_sb[g], OB_sb[g]
                for t0 in range(0, ntg, 3):
                    tn = min(3, ntg - t0)
                    pu = ps_u.tile([P, 512], f32, tag="u")
                    for t in range(t0, t0 + tn):
                        j = t - t0
                        for k in range(nkg):
                            nc.tensor.matmul(
                                pu[:, j * (DH + 1) : (j + 1) * (DH + 1)],
                                es[:, k, t * P : (t + 1) * P],
                                V_sb[:, k0 + k, :],
                                start=(k == 0), stop=(k == nkg - 1),
                            )
                    src = pu[:, : tn * (DH + 1)].rearrange(
                        "p (t f) -> p t f", t=tn)
                    if g == G - 1 and t0 > 0:
                        # ACT is idle after the last exp; let it evacuate the
                        # final psum group in parallel with DVE
                        nc.scalar.copy(ob[:, t0 : t0 + tn, :], src)
                    else:
                        nc.vector.tensor_copy(ob[:, t0 : t0 + tn, :], src)
                # stream this graph's rows out while later graphs compute
                if g < G - 1:
                    eng = nc.sync if g % 2 == 0 else nc.gpsimd
                    eng.dma_start(
                        out[a0 : a0 + ntg * P, :].rearrange(
                            "(t p) f -> p t f", p=P
                        ),
                        ob[:],
                    )
                else:
                    # last slot: head DMA needs only the first copy group;
                    # the exposed tail transfer is short
                    split = max(1, min(3, ntg - 1))
                    nc.sync.dma_start(
                        out[a0 : a0 + split * P, :].rearrange(
                            "(t p) f -> p t f", p=P
                        ),
                        ob[:, :split, :],
                    )
                    nc.scalar.dma_start(
                        out[a0 + split * P : a0 + ntg * P, :].rearrange(
                            "(t p) f -> p t f", p=P
                        ),
                        ob[:, split:, :],
                    )

            for g in range(G):
                emit_scores(g)
                emit_v(g)
                if g >= 1:
                    emit_u(g - 1)
            emit_u(G - 1)

    nc.compile()
    return nc


def _pack_slots(ac, rc, n_cores):
    """Group graphs into slots of ``n_cores`` similarly-shaped graphs.
    Returns (slots, assign) where slots[j] = (ntg, nkg) and assign[j] is the
    list of graph ids in slot j (one per core), ordered by slot cost desc."""
    a_ch = np.maximum(1, np.ceil(ac / P).astype(int))
    r_ch = np.maximum(1, np.ceil(rc / P).astype(int))
    from collections import defaultdict

    buckets = defaultdict(list)
    for g in range(len(ac)):
        buckets[(int(a_ch[g]), int(r_ch[g]))].append(g)

    slot_groups = []
    # pure same-shape slots first (cost order), then leftovers by cost
    for shape in sorted(buckets, key=lambda s: -(s[0] * s[1])):
        while len(buckets[shape]) >= n_cores:
            slot_groups.append([buckets[shape].pop() for _ in range(n_cores)])
    leftovers = [g for shape in sorted(buckets, key=lambda s: -(s[0] * s[1]))
                 for g in buckets[shape]]
    while leftovers:
        slot_groups.append(leftovers[:n_cores])
        leftovers = leftovers[n_cores:]
    # pad the final partial group by reusing empty pseudo-graphs (none here
    # for 32 graphs / 8 cores, but keep it safe)
    slots = []
    for grp in slot_groups:
        ntg = int(max(a_ch[g] for g in grp))
        nkg = int(max(r_ch[g] for g in grp))
        slots.append((ntg, nkg))
    # order slots by exp work desc so the cheapest slot is processed last
    order = sorted(range(len(slots)),
                   key=lambda j: -(slots[j][0] * slots[j][1]))
    slots = [slots[j] for j in order]
    slot_groups = [slot_groups[j] for j in order]
    return slots, slot_groups


def kernel(atom_h, residue_h, atom_batch, residue_batch, W_q, W_k, W_v):
    atom_h = np.asarray(atom_h, dtype=np.float32)
    residue_h = np.asarray(residue_h, dtype=np.float32)
    atom_batch = np.asarray(atom_batch)
    residue_batch = np.asarray(residue_batch)
    W_q = np.asarray(W_q, dtype=np.float32)
    W_k = np.asarray(W_k, dtype=np.float32)
    W_v = np.asarray(W_v, dtype=np.float32)

    A = atom_h.shape[0]
    R = residue_h.shape[0]
    n_b = max(B, int(atom_batch.max()) + 1 if A else B,
              int(residue_batch.max()) + 1 if R else B)

    ac = np.bincount(atom_batch, minlength=n_b)
    rc = np.bincount(residue_batch, minlength=n_b)
    a_off = np.concatenate([[0], np.cumsum(ac)])
    r_off = np.concatenate([[0], np.cumsum(rc)])

    slots, slot_groups = _pack_slots(ac, rc, N_CORES)
    G = len(slots)
    ntgs = [s[0] for s in slots]
    nkgs = [s[1] for s in slots]
    AGs = [t * P for t in ntgs]
    aoffs = np.concatenate([[0], np.cumsum(AGs)]).astype(int)
    koffs = np.concatenate([[0], np.cumsum(nkgs)]).astype(int)
    A_pad = int(aoffs[-1])
    nRc = int(koffs[-1])
    R_pad = nRc * P

    key = tuple(slots)
    if key not in _kernel_cache:
        _kernel_cache[key] = _build_kernel(key)
    nc = _kernel_cache[key]

    # host-side weight prep: fold W_q and the softmax scale into W_k
    wkq = ((W_k.T @ W_q) * SCALE).astype(BF16)        # [i, j]
    wvT0 = np.concatenate(
        [W_v.T, np.zeros((DH, 1), dtype=np.float32)], axis=1).astype(BF16)

    in_maps = []
    for c in range(N_CORES):
        atomT_c = np.zeros((P, A_pad), dtype=BF16)
        resT_c = np.zeros((P, R_pad), dtype=BF16)
        onesc_c = np.zeros((P, nRc), dtype=BF16)
        for j in range(G):
            if c >= len(slot_groups[j]):
                continue
            g = slot_groups[j][c]
            na, nr = int(ac[g]), int(rc[g])
            RGj = nkgs[j] * P
            if na:
                atomT_c[:, aoffs[j] : aoffs[j] + na] = (
                    atom_h[a_off[g] : a_off[g] + na].T.astype(BF16))
            if nr:
                resT_c[:, koffs[j] * P : koffs[j] * P + nr] = (
                    residue_h[r_off[g] : r_off[g] + nr].T.astype(BF16))
            flat = np.zeros(RGj, dtype=BF16)
            flat[:nr] = 1.0
            onesc_c[:, koffs[j] : koffs[j + 1]] = flat.reshape(nkgs[j], P).T
        head_c = np.concatenate(
            [wkq, resT_c[:, :P], wvT0, onesc_c], axis=1)
        in_maps.append({
            "atomT": atomT_c, "resT": resT_c, "head": head_c,
        })

    res = run_bass_kernel_spmd(nc, in_maps, core_ids=list(range(N_CORES)))

    result = atom_h.copy()
    for c in range(N_CORES):
        u = res.results[c]["out"]
        for j in range(G):
            if c >= len(slot_groups[j]):
                continue
            g = slot_groups[j][c]
            na, nr = int(ac[g]), int(rc[g])
            if na == 0 or nr == 0:
                continue
            rows = u[aoffs[j] : aoffs[j] + na]
            result[a_off[g] : a_off[g] + na] += rows[:, :DH] / rows[:, DH : DH + 1]
    return result


# revision 49
# speedup vs baseline: 1.6849x; 1.0491x over previous
"""Cross-graph attention (block-diagonal segment-local attention) on 8 trn2 cores.

Strategy: graphs are contiguous segments in the sorted batch arrays; attention
is block-diagonal.  4 graphs per core; the 32 graphs are grouped host-side
into 4 slot-groups of 8 similarly-sized graphs (one per core), and each slot
has its own padded (AG_j, RG_j) so all 8 cores run one identical SPMD program
with minimal padding.

Design:
  - W_q is folded into W_k host-side (W_kq = W_k^T W_q / sqrt(DH)):
    scores^T = T1^T-contraction with raw atom columns; no Q stage on device.
  - everything streams in bf16 (half the DMA bytes; matmuls run at
    1 cycle/row at any output width, so V/U are exactly 129 wide).
  - masking via the V ones-column: padded residues get V rows = 0 and
    ones-col = 0 (host-packed), so their exp(0)=1 contributes nothing to
    numerator or denominator.  No ACT bias, so exp instructions merge
    across residue chunks (one ACT per <=2 chunks).
  - a dummy warmup matmul at t~0 starts the PE p-state ramp clock.
  - U/V psum outputs are packed 3-per-bank; DVE evacuates all PSUM
    (Pool/GPSIMD cannot touch PSUM on HW); ACT does only the exp.
  - output DMAs are spread across SP / Pool(SWDGE) / ACT queues; the last
    slot is the cheapest one and its output is split so the exposed tail
    transfer is short.
  - normalization + residual add run host-side: out = atom_h + U[:, :128]/U[:, 128].
"""

import sys

if "/opt/trn_rl_repo" not in sys.path:
    sys.path.insert(0, "/opt/trn_rl_repo")

import ml_dtypes
import numpy as np

import concourse.bass as bass
import concourse.tile as tile
from concourse import bacc, mybir
from concourse.bass_utils import run_bass_kernel_spmd

N_CORES = 8
B = 32                      # number of graphs
P = 128                     # partitions
DH = 128                    # feature dims (DA == DR == DH == 128)
SCALE = 1.0 / np.sqrt(128.0)

BF16 = ml_dtypes.bfloat16

_kernel_cache: dict = {}


def _bank_chunks(lo, hi):
    """Split [lo, hi) into matmul chunks that never cross a 512-col PSUM
    bank boundary."""
    out, i = [], lo
    while i < hi:
        nxt = min(hi, (i // 512 + 1) * 512)
        out.append((i, nxt - i))
        i = nxt
    return out


def _build_kernel(slots):
    """One SPMD program; ``slots`` is a tuple of (ntg, nkg) per graph slot:
    slot j holds one graph of <= ntg*128 atoms / nkg*128 residues."""
    G = len(slots)
    AGs = [s[0] for s in slots]          # 64-multiples
    nkgs = [s[1] for s in slots]
    ntgs = [(a + P - 1) // P for a in AGs]
    aoffs = np.concatenate([[0], np.cumsum(AGs)]).astype(int)
    ooffs = np.concatenate([[0], np.cumsum([t * P for t in ntgs])]).astype(int)
    koffs = np.concatenate([[0], np.cumsum(nkgs)]).astype(int)
    A_pad = int(aoffs[-1])
    O_pad = int(ooffs[-1])
    nRc = int(koffs[-1])
    R_pad = nRc * P
    R0 = nkgs[0] * P                     # graph-0 residues ride in the head
    f32 = mybir.dt.float32
    bf16 = mybir.dt.bfloat16

    nc = bacc.Bacc("TRN2")
    # head = [wkq(128) | graph-0 residue chunks | wvT+zerocol(129) | onesc]
    HW_ = DH + R0 + (DH + 1) + nRc
    atomT = nc.dram_tensor("atomT", [P, A_pad], bf16, kind="ExternalInput")
    resT = nc.dram_tensor("resT", [P, R_pad], bf16, kind="ExternalInput")
    head = nc.dram_tensor("head", [P, HW_], bf16, kind="ExternalInput")
    out = nc.dram_tensor("out", [O_pad, DH + 1], f32, kind="ExternalOutput")

    with tile.TileContext(nc) as tc:
        with (
            tc.tile_pool(name="singles", bufs=1) as singles,
            tc.tile_pool(name="ps_sc", bufs=3, space="PSUM") as ps_sc,
            tc.tile_pool(name="ps_u", bufs=2, space="PSUM") as ps_u,
        ):
            atomT_sb = singles.tile([P, A_pad], bf16)
            resT_sb = singles.tile([P, R_pad], bf16)
            head_sb = singles.tile([P, HW_], bf16)
            T1T_sb = singles.tile([P, R_pad], bf16)
            V_sb = singles.tile([P, nRc, DH + 1], bf16)
            warm_sb = singles.tile([P, 2], bf16)
            wkq_sb = head_sb[:, :DH]
            wvT_sb = head_sb[:, DH + R0 : DH + R0 + DH + 1]
            onesc_sb = head_sb[:, DH + R0 + DH + 1 :]

            def res_chunk(kg):
                # graph-0 residue chunks ride in the head DMA
                if kg * P < R0:
                    return head_sb[:, DH + kg * P : DH + (kg + 1) * P]
                return resT_sb[:, kg * P : (kg + 1) * P]
            ES_sb = [singles.tile([P, nkgs[g], AGs[g]], bf16, name=f"es{g}")
                     for g in range(G)]
            OB_sb = [singles.tile([P, ntgs[g], DH + 1], f32, name=f"ob{g}")
                     for g in range(G)]

            # ---- PE warmup: start the p-state ramp clock immediately ----
            nc.gpsimd.memset(warm_sb[:], 0.0)
            pw = ps_u.tile([P, 512], f32, tag="u")
            nc.tensor.matmul(pw[:2, :1], warm_sb[:], warm_sb[:, :1],
                             start=True, stop=True)

            # ---- input DMAs (SP/HWDGE), in first-use order ----
            nc.sync.dma_start(head_sb[:], head[:])
            nc.sync.dma_start(atomT_sb[:, : AGs[0]], atomT[:, : AGs[0]])
            if R_pad > R0:
                nc.sync.dma_start(resT_sb[:, R0 : min(R0 + 512, R_pad)],
                                  resT[:, R0 : min(R0 + 512, R_pad)])
            if R_pad > R0 + 512:
                nc.sync.dma_start(resT_sb[:, R0 + 512 :],
                                  resT[:, R0 + 512 :])
            nc.sync.dma_start(atomT_sb[:, AGs[0] :], atomT[:, AGs[0] :])
            # (atom-rest is last: graph-1+ scores start well after it lands)

            # V ones-column (0 for padded residues), written once; V copies
            # only write the first DH columns so this is never clobbered
            nc.gpsimd.tensor_copy(V_sb[:, :, DH], onesc_sb)

            # ---- T1^T = contraction of W_kq with res^T ----
            # Piece list: per-residue-chunk pieces for graph 0 (so each
            # scores k starts as soon as its own residues land), then
            # 512-col chunks.  Emitted lazily via ensure_t1t.
            pieces = [(kg * P, P) for kg in range(nkgs[0])]
            i = R0
            while i < R_pad:
                w = min(512, R_pad - i)
                pieces.append((i, w))
                i += w
            t1_next = [0]    # next piece index to emit

            def ensure_t1t(upto):
                """Emit T1T pieces until residue columns [0, upto) covered."""
                while t1_next[0] < len(pieces):
                    lo, w = pieces[t1_next[0]]
                    if lo >= upto:
                        break
                    src = res_chunk(lo // P) if lo < R0 else \
                        resT_sb[:, lo : lo + w]
                    pt = ps_u.tile([P, 512], f32, tag="u")
                    nc.tensor.matmul(pt[:, :w], wkq_sb, src,
                                     start=True, stop=True)
                    nc.vector.tensor_copy(T1T_sb[:, lo : lo + w], pt[:, :w])
                    t1_next[0] += 1

            # ---- V rows per graph, packed 3 per psum bank ----
            def emit_v(g):
                k0, nkg = int(koffs[g]), nkgs[g]
                pv = ps_u.tile([P, 512], f32, tag="u")
                for j in range(nkg):
                    nc.tensor.matmul(pv[:, j * (DH + 1) : (j + 1) * (DH + 1)],
                                     res_chunk(k0 + j),
                                     wvT_sb, start=True, stop=True)
                pvv = pv[:, : nkg * (DH + 1)].rearrange(
                    "p (k f) -> p k f", k=nkg)
                # copy only the value columns; the ones-column was written
                # once upfront and must not be clobbered
                nc.vector.tensor_copy(V_sb[:, k0 : k0 + nkg, :DH],
                                      pvv[:, :, :DH])

            # ---- per-graph attention, ACT-paced pipeline ----
            def emit_scores(g):
                a0, k0 = int(aoffs[g]), int(koffs[g])
                AG, nkg = AGs[g], nkgs[g]
                es = ES_sb[g]
                for k in range(nkg):
                    # stay 3 residue chunks ahead so T1T copies are never on
                    # the scores critical path
                    ensure_t1t(min(R_pad, (k0 + k + 3) * P))
                    ps = ps_sc.tile([P, 640], f32, tag="sc")
                    for c, w in _bank_chunks(0, AG):
                        nc.tensor.matmul(
                            ps[:, c : c + w],
                            T1T_sb[:, (k0 + k) * P : (k0 + k + 1) * P],
                            atomT_sb[:, a0 + c : a0 + c + w],
                            start=True, stop=True,
                        )
                    nc.scalar.activation(
                        es[:, k, :], ps[:, :AG],
                        mybir.ActivationFunctionType.Exp,
                    )

            def emit_u(g):
                a0, k0 = int(ooffs[g]), int(koffs[g])
                AG, ntg, nkg = AGs[g], ntgs[g], nkgs[g]
                es, ob = ES_sb[g], OB_sb[g]
                last = g == G - 1
                gsz = 2 if last else 3
                dmas = [nc.sync, nc.scalar, nc.gpsimd]
                for gi, t0 in enumerate(range(0, ntg, gsz)):
                    tn = min(gsz, ntg - t0)
                    pu = ps_u.tile([P, 512], f32, tag="u")
                    for t in range(t0, t0 + tn):
                        j = t - t0
                        tw = min(P, AG - t * P)
                        for k in range(nkg):
                            nc.tensor.matmul(
                                pu[:tw, j * (DH + 1) : (j + 1) * (DH + 1)],
                                es[:, k, t * P : t * P + tw],
                                V_sb[:, k0 + k, :],
                                start=(k == 0), stop=(k == nkg - 1),
                            )
                    src = pu[:, : tn * (DH + 1)].rearrange(
                        "p (t f) -> p t f", t=tn)
                    if last and gi % 2 == 1:
                        # ACT is idle after the last exp; let it help DVE
                        # evacuate the final psum groups in parallel
                        nc.scalar.copy(ob[:, t0 : t0 + tn, :], src)
                    else:
                        nc.vector.tensor_copy(ob[:, t0 : t0 + tn, :], src)
                    if last:
                        # stream each piece out on its own queue so the
                        # exposed tail transfer is short
                        dmas[gi % 3].dma_start(
                            out[a0 + t0 * P : a0 + (t0 + tn) * P, :].rearrange(
                                "(t p) f -> p t f", p=P
                            ),
                            ob[:, t0 : t0 + tn, :],
                        )
                if not last:
                    # stream this graph's rows out while later graphs
                    # compute (never on ACT: a queued DMA SEQ wait would
                    # stall later exps)
                    eng = nc.sync if g % 2 == 0 else nc.gpsimd
                    eng.dma_start(
                        out[a0 : a0 + ntg * P, :].rearrange(
                            "(t p) f -> p t f", p=P
                        ),
                        ob[:],
                    )

            for g in range(G):
                emit_scores(g)
                emit_v(g)
                if g >= 1:
                    emit_u(g - 1)
            emit_u(G - 1)

    nc.compile()
    return nc


def _pack_slots(ac, rc, n_cores):
    """Group graphs into slots of ``n_cores`` similarly-shaped graphs.
    Returns (slots, assign) where slots[j] = (AG, nkg) — AG a 64-multiple —
    and assign[j] is the list of graph ids in slot j (one per core),
    ordered by slot cost desc (cheapest slot processed last)."""
    a_ch = np.maximum(1, np.ceil(ac / P).astype(int))
    r_ch = np.maximum(1, np.ceil(rc / P).astype(int))
    from collections import defaultdict

    buckets = defaultdict(list)
    for g in range(len(ac)):
        buckets[(int(a_ch[g]), int(r_ch[g]))].append(g)
    # within a bucket, pure slots pop the largest graphs; the smallest
    # leak into the mixed leftover slot
    for shape in buckets:
        buckets[shape].sort(key=lambda g: int(ac[g]))

    slot_groups = []
    for shape in sorted(buckets, key=lambda s: -(s[0] * s[1])):
        while len(buckets[shape]) >= n_cores:
            slot_groups.append([buckets[shape].pop() for _ in range(n_cores)])
    leftovers = [g for shape in sorted(buckets, key=lambda s: -(s[0] * s[1]))
                 for g in buckets[shape]]
    while leftovers:
        slot_groups.append(leftovers[:n_cores])
        leftovers = leftovers[n_cores:]
    slots = []
    for grp in slot_groups:
        amax = int(max(ac[g] for g in grp))
        nkg = int(max(r_ch[g] for g in grp))
        AG = max(P, (amax + 63) // 64 * 64)
        slots.append((AG, nkg))
    # order slots by exp work desc so the cheapest slot is processed last
    order = sorted(range(len(slots)),
                   key=lambda j: -(slots[j][0] * slots[j][1]))
    slots = [slots[j] for j in order]
    slot_groups = [slot_groups[j] for j in order]
    return slots, slot_groups


def kernel(atom_h, residue_h, atom_batch, residue_batch, W_q, W_k, W_v):
    atom_h = np.asarray(atom_h, dtype=np.float32)
    residue_h = np.asarray(residue_h, dtype=np.float32)
    atom_batch = np.asarray(atom_batch)
    residue_batch = np.asarray(residue_batch)
    W_q = np.asarray(W_q, dtype=np.float32)
    W_k = np.asarray(W_k, dtype=np.float32)
    W_v = np.asarray(W_v, dtype=np.float32)

    A = atom_h.shape[0]
    R = residue_h.shape[0]
    n_b = max(B, int(atom_batch.max()) + 1 if A else B,
              int(residue_batch.max()) + 1 if R else B)

    ac = np.bincount(atom_batch, minlength=n_b)
    rc = np.bincount(residue_batch, minlength=n_b)
    a_off = np.concatenate([[0], np.cumsum(ac)])
    r_off = np.concatenate([[0], np.cumsum(rc)])

    slots, slot_groups = _pack_slots(ac, rc, N_CORES)
    G = len(slots)
    AGs = [s[0] for s in slots]
    nkgs = [s[1] for s in slots]
    ntgs = [(a + P - 1) // P for a in AGs]
    aoffs = np.concatenate([[0], np.cumsum(AGs)]).astype(int)
    ooffs = np.concatenate([[0], np.cumsum([t * P for t in ntgs])]).astype(int)
    koffs = np.concatenate([[0], np.cumsum(nkgs)]).astype(int)
    A_pad = int(aoffs[-1])
    nRc = int(koffs[-1])
    R_pad = nRc * P

    key = tuple(slots)
    if key not in _kernel_cache:
        _kernel_cache[key] = _build_kernel(key)
    nc = _kernel_cache[key]

    # host-side weight prep: fold W_q and the softmax scale into W_k
    wkq = ((W_k.T @ W_q) * SCALE).astype(BF16)        # [i, j]
    wvT0 = np.concatenate(
        [W_v.T, np.zeros((DH, 1), dtype=np.float32)], axis=1).astype(BF16)

    in_maps = []
    for c in range(N_CORES):
        atomT_c = np.zeros((P, A_pad), dtype=BF16)
        resT_c = np.zeros((P, R_pad), dtype=BF16)
        onesc_c = np.zeros((P, nRc), dtype=BF16)
        for j in range(G):
            if c >= len(slot_groups[j]):
                continue
            g = slot_groups[j][c]
            na, nr = int(ac[g]), int(rc[g])
            RGj = nkgs[j] * P
            if na:
                atomT_c[:, aoffs[j] : aoffs[j] + na] = (
                    atom_h[a_off[g] : a_off[g] + na].T.astype(BF16))
            if nr:
                resT_c[:, koffs[j] * P : koffs[j] * P + nr] = (
                    residue_h[r_off[g] : r_off[g] + nr].T.astype(BF16))
            flat = np.zeros(RGj, dtype=BF16)
            flat[:nr] = 1.0
            onesc_c[:, koffs[j] : koffs[j + 1]] = flat.reshape(nkgs[j], P).T
        head_c = np.concatenate(
            [wkq, resT_c[:, : nkgs[0] * P], wvT0, onesc_c], axis=1)
        in_maps.append({
            "atomT": atomT_c, "resT": resT_c, "head": head_c,
        })

    res = run_bass_kernel_spmd(nc, in_maps, core_ids=list(range(N_CORES)))

    result = atom_h.copy()
    for c in range(N_CORES):
        u = res.results[c]["out"]
        for j in range(G):
            if c >= len(slot_groups[j]):
                continue
            g = slot_groups[j][c]
            na, nr = int(ac[g]), int(rc[g])
            if na == 0 or nr == 0:
                continue
            rows = u[ooffs[j] : ooffs[j] + na]
            result[a_off[g] : a_off[g] + na] += rows[:, :DH] / rows[:, DH : DH + 1]
    return result


# revision 52
# speedup vs baseline: 1.7984x; 1.0674x over previous
"""Cross-graph attention (block-diagonal segment-local attention) on 8 trn2 cores.

Strategy: graphs are contiguous segments in the sorted batch arrays; attention
is block-diagonal.  4 graphs per core; the 32 graphs are grouped host-side
into 4 slot-groups of 8 similarly-sized graphs (one per core), and each slot
has its own padded (AG_j, RG_j) so all 8 cores run one identical SPMD program
with minimal padding.

Design:
  - W_q is folded into W_k host-side (W_kq = W_k^T W_q / sqrt(DH)):
    scores^T = T1^T-contraction with raw atom columns; no Q stage on device.
  - everything streams in bf16 (half the DMA bytes; matmuls run at
    1 cycle/row at any output width, so V/U are exactly 129 wide).
  - masking via the V ones-column: padded residues get V rows = 0 and
    ones-col = 0 (host-packed), so their exp(0)=1 contributes nothing to
    numerator or denominator.  No ACT bias, so exp instructions merge
    across residue chunks (one ACT per <=2 chunks).
  - a dummy warmup matmul at t~0 starts the PE p-state ramp clock.
  - U/V psum outputs are packed 3-per-bank; DVE evacuates all PSUM
    (Pool/GPSIMD cannot touch PSUM on HW); ACT does only the exp.
  - output DMAs are spread across SP / Pool(SWDGE) / ACT queues; the last
    slot is the cheapest one and its output is split so the exposed tail
    transfer is short.
  - normalization + residual add run host-side: out = atom_h + U[:, :128]/U[:, 128].
"""

import sys

if "/opt/trn_rl_repo" not in sys.path:
    sys.path.insert(0, "/opt/trn_rl_repo")

import ml_dtypes
import numpy as np

import concourse.bass as bass
import concourse.tile as tile
from concourse import bacc, mybir
from concourse.bass_utils import run_bass_kernel_spmd

N_CORES = 8
B = 32                      # number of graphs
P = 128                     # partitions
DH = 128                    # feature dims (DA == DR == DH == 128)
SCALE = 1.0 / np.sqrt(128.0)

BF16 = ml_dtypes.bfloat16

_kernel_cache: dict = {}


def _bank_chunks(lo, hi):
    """Split [lo, hi) into matmul chunks that never cross a 512-col PSUM
    bank boundary."""
    out, i = [], lo
    while i < hi:
        nxt = min(hi, (i // 512 + 1) * 512)
        out.append((i, nxt - i))
        i = nxt
    return out


def _build_kernel(slots):
    """One SPMD program; ``slots`` is a tuple of (ntg, nkg) per graph slot:
    slot j holds one graph of <= ntg*128 atoms / nkg*128 residues."""
    G = len(slots)
    AGs = [s[0] for s in slots]          # 64-multiples
    nkgs = [s[1] for s in slots]
    ntgs = [(a + P - 1) // P for a in AGs]
    aoffs = np.concatenate([[0], np.cumsum(AGs)]).astype(int)
    ooffs = np.concatenate([[0], np.cumsum([t * P for t in ntgs])]).astype(int)
    koffs = np.concatenate([[0], np.cumsum(nkgs)]).astype(int)
    A_pad = int(aoffs[-1])
    O_pad = int(ooffs[-1])
    nRc = int(koffs[-1])
    R_pad = nRc * P
    R0 = nkgs[0] * P                     # graph-0 residues ride in the head
    f32 = mybir.dt.float32
    bf16 = mybir.dt.bfloat16

    nc = bacc.Bacc("TRN2")
    # head = [wkq(128) | graph-0 residue chunks | wvT+zerocol(129) | onesc]
    HW_ = DH + R0 + (DH + 1) + nRc
    atomT = nc.dram_tensor("atomT", [P, A_pad], bf16, kind="ExternalInput")
    resT = nc.dram_tensor("resT", [P, R_pad], bf16, kind="ExternalInput")
    head = nc.dram_tensor("head", [P, HW_], bf16, kind="ExternalInput")
    out = nc.dram_tensor("out", [O_pad, DH + 1], f32, kind="ExternalOutput")

    with tile.TileContext(nc) as tc:
        with (
            tc.tile_pool(name="singles", bufs=1) as singles,
            tc.tile_pool(name="ps_sc", bufs=2, space="PSUM") as ps_sc,
            tc.tile_pool(name="ps_u", bufs=3, space="PSUM") as ps_u,
        ):
            atomT_sb = singles.tile([P, A_pad], bf16)
            resT_sb = singles.tile([P, R_pad], bf16)
            head_sb = singles.tile([P, HW_], bf16)
            T1T_sb = singles.tile([P, R_pad], bf16)
            V_sb = singles.tile([P, nRc, DH + 1], bf16)
            warm_sb = singles.tile([P, 2], bf16)
            wkq_sb = head_sb[:, :DH]
            wvT_sb = head_sb[:, DH + R0 : DH + R0 + DH + 1]
            onesc_sb = head_sb[:, DH + R0 + DH + 1 :]

            def res_chunk(kg):
                # graph-0 residue chunks ride in the head DMA
                if kg * P < R0:
                    return head_sb[:, DH + kg * P : DH + (kg + 1) * P]
                return resT_sb[:, kg * P : (kg + 1) * P]
            ES_sb = [singles.tile([P, nkgs[g], AGs[g]], bf16, name=f"es{g}")
                     for g in range(G)]
            OB_sb = [singles.tile([P, ntgs[g], DH + 1], f32, name=f"ob{g}")
                     for g in range(G)]

            # ---- PE warmup: start the p-state ramp clock immediately ----
            nc.gpsimd.memset(warm_sb[:], 0.0)
            pw = ps_u.tile([P, 512], f32, tag="u")
            nc.tensor.matmul(pw[:2, :1], warm_sb[:], warm_sb[:, :1],
                             start=True, stop=True)

            # ---- input DMAs (SP/HWDGE), in first-use order ----
            a1 = int(aoffs[1]) if G > 1 else A_pad
            a2 = int(aoffs[2]) if G > 2 else A_pad
            nc.sync.dma_start(head_sb[:], head[:])
            nc.sync.dma_start(atomT_sb[:, :a1], atomT[:, :a1])
            if R_pad > R0:
                nc.sync.dma_start(resT_sb[:, R0 : min(R0 + 512, R_pad)],
                                  resT[:, R0 : min(R0 + 512, R_pad)])
            if a2 > a1:
                nc.sync.dma_start(atomT_sb[:, a1:a2], atomT[:, a1:a2])
            if R_pad > R0 + 512:
                nc.sync.dma_start(resT_sb[:, R0 + 512 :],
                                  resT[:, R0 + 512 :])
            if A_pad > a2:
                nc.sync.dma_start(atomT_sb[:, a2:], atomT[:, a2:])
            # (later atom slots last: their scores start well after they land)

            # V ones-column (0 for padded residues), written once; V copies
            # only write the first DH columns so this is never clobbered
            nc.gpsimd.tensor_copy(V_sb[:, :, DH], onesc_sb)

            # ---- T1^T = contraction of W_kq with res^T ----
            # Piece list: per-residue-chunk pieces for graph 0 (so each
            # scores k starts as soon as its own residues land), then
            # 512-col chunks.  Emitted lazily via ensure_t1t.
            pieces = [(kg * P, P) for kg in range(nkgs[0])]
            i = R0
            while i < R_pad:
                w = min(512, R_pad - i)
                pieces.append((i, w))
                i += w
            t1_next = [0]    # next piece index to emit

            def ensure_t1t(upto):
                """Emit T1T pieces until residue columns [0, upto) covered."""
                while t1_next[0] < len(pieces):
                    lo, w = pieces[t1_next[0]]
                    if lo >= upto:
                        break
                    src = res_chunk(lo // P) if lo < R0 else \
                        resT_sb[:, lo : lo + w]
                    pt = ps_u.tile([P, 512], f32, tag="u")
                    nc.tensor.matmul(pt[:, :w], wkq_sb, src,
                                     start=True, stop=True)
                    nc.vector.tensor_copy(T1T_sb[:, lo : lo + w], pt[:, :w])
                    t1_next[0] += 1

            # ---- V rows per graph, packed 3 per psum bank ----
            def emit_v(g):
                k0, nkg = int(koffs[g]), nkgs[g]
                pv = ps_u.tile([P, 512], f32, tag="u")
                for j in range(nkg):
                    nc.tensor.matmul(pv[:, j * (DH + 1) : (j + 1) * (DH + 1)],
                                     res_chunk(k0 + j),
                                     wvT_sb, start=True, stop=True)
                pvv = pv[:, : nkg * (DH + 1)].rearrange(
                    "p (k f) -> p k f", k=nkg)
                # copy only the value columns; the ones-column was written
                # once upfront and must not be clobbered
                nc.vector.tensor_copy(V_sb[:, k0 : k0 + nkg, :DH],
                                      pvv[:, :, :DH])

            # ---- per-graph attention, ACT-paced pipeline ----
            def emit_scores(g):
                a0, k0 = int(aoffs[g]), int(koffs[g])
                AG, nkg = AGs[g], nkgs[g]
                es = ES_sb[g]
                for k in range(nkg):
                    # stay 3 residue chunks ahead so T1T copies are never on
                    # the scores critical path
                    ensure_t1t(min(R_pad, (k0 + k + 3) * P))
                    ps = ps_sc.tile([P, 640], f32, tag="sc")
                    for c, w in _bank_chunks(0, AG):
                        nc.tensor.matmul(
                            ps[:, c : c + w],
                            T1T_sb[:, (k0 + k) * P : (k0 + k + 1) * P],
                            atomT_sb[:, a0 + c : a0 + c + w],
                            start=True, stop=True,
                        )
                    nc.scalar.activation(
                        es[:, k, :], ps[:, :AG],
                        mybir.ActivationFunctionType.Exp,
                    )

            def emit_u(g):
                a0, k0 = int(ooffs[g]), int(koffs[g])
                AG, ntg, nkg = AGs[g], ntgs[g], nkgs[g]
                es, ob = ES_sb[g], OB_sb[g]
                last = g == G - 1
                gsz = 2 if last else 3
                dmas = [nc.gpsimd, nc.sync, nc.scalar]
                for gi, t0 in enumerate(range(0, ntg, gsz)):
                    tn = min(gsz, ntg - t0)
                    pu = ps_u.tile([P, 512], f32, tag="u")
                    for t in range(t0, t0 + tn):
                        j = t - t0
                        tw = min(P, AG - t * P)
                        for k in range(nkg):
                            nc.tensor.matmul(
                                pu[:tw, j * (DH + 1) : (j + 1) * (DH + 1)],
                                es[:, k, t * P : t * P + tw],
                                V_sb[:, k0 + k, :],
                                start=(k == 0), stop=(k == nkg - 1),
                            )
                    src = pu[:, : tn * (DH + 1)].rearrange(
                        "p (t f) -> p t f", t=tn)
                    if last and gi % 2 == 1:
                        # ACT is idle after the last exp; let it help DVE
                        # evacuate the final psum groups in parallel
                        nc.scalar.copy(ob[:, t0 : t0 + tn, :], src)
                    else:
                        nc.vector.tensor_copy(ob[:, t0 : t0 + tn, :], src)
                    if last:
                        # stream each piece out on its own queue so the
                        # exposed tail transfer is short
                        dmas[gi % 3].dma_start(
                            out[a0 + t0 * P : a0 + (t0 + tn) * P, :].rearrange(
                                "(t p) f -> p t f", p=P
                            ),
                            ob[:, t0 : t0 + tn, :],
                        )
                if not last:
                    # stream this graph's rows out while later graphs
                    # compute (never on ACT: a queued DMA SEQ wait would
                    # stall later exps)
                    eng = nc.sync if g % 2 == 0 else nc.gpsimd
                    eng.dma_start(
                        out[a0 : a0 + ntg * P, :].rearrange(
                            "(t p) f -> p t f", p=P
                        ),
                        ob[:],
                    )

            for g in range(G):
                emit_scores(g)
                if g >= 1:
                    emit_u(g - 1)
                emit_v(g)
            emit_u(G - 1)

    nc.compile()
    return nc


def _pack_slots(ac, rc, n_cores):
    """Group graphs into slots of ``n_cores`` similarly-shaped graphs.
    Returns (slots, assign) where slots[j] = (AG, nkg) — AG a 64-multiple —
    and assign[j] is the list of graph ids in slot j (one per core),
    ordered by slot cost desc (cheapest slot processed last)."""
    a_ch = np.maximum(1, np.ceil(ac / P).astype(int))
    r_ch = np.maximum(1, np.ceil(rc / P).astype(int))
    from collections import defaultdict

    buckets = defaultdict(list)
    for g in range(len(ac)):
        buckets[(int(a_ch[g]), int(r_ch[g]))].append(g)
    # within a bucket, pure slots pop the largest graphs; the smallest
    # leak into the mixed leftover slot
    for shape in buckets:
        buckets[shape].sort(key=lambda g: int(ac[g]))

    slot_groups = []
    for shape in sorted(buckets, key=lambda s: -(s[0] * s[1])):
        while len(buckets[shape]) >= n_cores:
            slot_groups.append([buckets[shape].pop() for _ in range(n_cores)])
    leftovers = [g for shape in sorted(buckets, key=lambda s: -(s[0] * s[1]))
                 for g in buckets[shape]]
    while leftovers:
        slot_groups.append(leftovers[:n_cores])
        leftovers = leftovers[n_cores:]
    slots = []
    for grp in slot_groups:
        amax = int(max(ac[g] for g in grp))
        nkg = int(max(r_ch[g] for g in grp))
        AG = max(P, (amax + 63) // 64 * 64)
        slots.append((AG, nkg))
    # order slots by exp work desc so the cheapest slot is processed last
    order = sorted(range(len(slots)),
                   key=lambda j: -(slots[j][0] * slots[j][1]))
    slots = [slots[j] for j in order]
    slot_groups = [slot_groups[j] for j in order]
    return slots, slot_groups


def kernel(atom_h, residue_h, atom_batch, residue_batch, W_q, W_k, W_v):
    atom_h = np.asarray(atom_h, dtype=np.float32)
    residue_h = np.asarray(residue_h, dtype=np.float32)
    atom_batch = np.asarray(atom_batch)
    residue_batch = np.asarray(residue_batch)
    W_q = np.asarray(W_q, dtype=np.float32)
    W_k = np.asarray(W_k, dtype=np.float32)
    W_v = np.asarray(W_v, dtype=np.float32)

    A = atom_h.shape[0]
    R = residue_h.shape[0]
    n_b = max(B, int(atom_batch.max()) + 1 if A else B,
              int(residue_batch.max()) + 1 if R else B)

    ac = np.bincount(atom_batch, minlength=n_b)
    rc = np.bincount(residue_batch, minlength=n_b)
    a_off = np.concatenate([[0], np.cumsum(ac)])
    r_off = np.concatenate([[0], np.cumsum(rc)])

    slots, slot_groups = _pack_slots(ac, rc, N_CORES)
    G = len(slots)
    AGs = [s[0] for s in slots]
    nkgs = [s[1] for s in slots]
    ntgs = [(a + P - 1) // P for a in AGs]
    aoffs = np.concatenate([[0], np.cumsum(AGs)]).astype(int)
    ooffs = np.concatenate([[0], np.cumsum([t * P for t in ntgs])]).astype(int)
    koffs = np.concatenate([[0], np.cumsum(nkgs)]).astype(int)
    A_pad = int(aoffs[-1])
    nRc = int(koffs[-1])
    R_pad = nRc * P

    key = tuple(slots)
    if key not in _kernel_cache:
        _kernel_cache[key] = _build_kernel(key)
    nc = _kernel_cache[key]

    # host-side weight prep: fold W_q and the softmax scale into W_k
    wkq = ((W_k.T @ W_q) * SCALE).astype(BF16)        # [i, j]
    wvT0 = np.concatenate(
        [W_v.T, np.zeros((DH, 1), dtype=np.float32)], axis=1).astype(BF16)

    in_maps = []
    for c in range(N_CORES):
        atomT_c = np.zeros((P, A_pad), dtype=BF16)
        resT_c = np.zeros((P, R_pad), dtype=BF16)
        onesc_c = np.zeros((P, nRc), dtype=BF16)
        for j in range(G):
            if c >= len(slot_groups[j]):
                continue
            g = slot_groups[j][c]
            na, nr = int(ac[g]), int(rc[g])
            RGj = nkgs[j] * P
            if na:
                atomT_c[:, aoffs[j] : aoffs[j] + na] = (
                    atom_h[a_off[g] : a_off[g] + na].T.astype(BF16))
            if nr:
                resT_c[:, koffs[j] * P : koffs[j] * P + nr] = (
                    residue_h[r_off[g] : r_off[g] + nr].T.astype(BF16))
            flat = np.zeros(RGj, dtype=BF16)
            flat[:nr] = 1.0
            onesc_c[:, koffs[j] : koffs[j + 1]] = flat.reshape(nkgs[j], P).T
        head_c = np.concatenate(
            [wkq, resT_c[:, : nkgs[0] * P], wvT0, onesc_c], axis=1)
        in_maps.append({
            "atomT": atomT_c, "resT": resT_c, "head": head_c,
        })

    res = run_bass_kernel_spmd(nc, in_maps, core_ids=list(range(N_CORES)))

    result = atom_h.copy()
    for c in range(N_CORES):
        u = res.results[c]["out"]
        for j in range(G):
            if c >= len(slot_groups[j]):
                continue
            g = slot_groups[j][c]
            na, nr = int(ac[g]), int(rc[g])
            if na == 0 or nr == 0:
                continue
            rows = u[ooffs[j] : ooffs[j] + na]
            result[a_off[g] : a_off[g] + na] += rows[:, :DH] / rows[:, DH : DH + 1]
    return result
